# revision 3
# baseline (speedup 1.0000x reference)
"""Bass/Tile TRN2 kernel for nn_Attention (Bahdanau-style attention scores).

Computation (per batch b):
    energy[s, h] = tanh( (enc[b] @ We)[s, h] + (hidden[b] @ Wh)[h] + bias[h] )
    scores[s]    = sum_h energy[s, h] * v[h]
    out[b]       = softmax(scores)

Sharding: data-parallel over batch B=32 across 8 cores (4 batches/core);
W, b, v replicated.

Per-core device program:
  - enc is transposed to [e, s] layout ON THE HOST and packed so every
    SBUF partition line is one contiguous 16KB DMA descriptor (the old
    device-side DMA-transpose was the bottleneck: 32768 x 256B descriptors
    kept the DMA engines 95% busy and paced the whole kernel).
  - main matmul We-tile @ encT in bf16 (1 cyc/row, fp32 PSUM accumulate),
    output layout [h, s] so the (h@Wh + b) bias is a per-partition scalar
    fused into the ScalarE tanh.
  - v-dot as a k=h matmul with v as a [128,1] stationary (f32r).
  - chunk tails (tanh + v-dot) are emitted one chunk behind the main
    matmuls so the PE never waits on ScalarE; 5 PSUM banks rotate for
    the main GEMM (+2 score banks +1 bias-setup bank = 8).
  - softmax over s on partition 0 (reduce_max -> exp with fused sum -> mul).
"""

import ml_dtypes
import numpy as np

import concourse.bass as bass
import concourse.tile as tile
from concourse import bacc, mybir
from concourse import bass_utils
from concourse.masks import make_identity

F32 = mybir.dt.float32
F32R = mybir.dt.float32r
BF16 = mybir.dt.bfloat16
AFT = mybir.ActivationFunctionType
AXX = mybir.AxisListType.X

N_CORES = 8
B = 32
B_LOC = B // N_CORES  # 4
S = 1024
H = 512
E2 = 2 * H  # 1024
P = 128
N_HT = H // P   # 4 h-tiles
N_ET = E2 // P  # 8 e-tiles
N_SC = S // 512  # 2 s-chunks of 512
SM_C = H + B_LOC + 2


def build():
    nc = bacc.Bacc("TRN2", target_bir_lowering=False, debug=False)
    # host-pretransposed/packed: enc[b, p, j*S + s] = enc_orig[b, s, j*128+p]
    enc = nc.dram_tensor("enc", [B_LOC, P, N_ET * S], BF16, kind="ExternalInput").ap()
    # host-packed: We[p, j*H + h] = We_orig[j*128+p, h]
    We_d = nc.dram_tensor("We", [P, N_ET * H], BF16, kind="ExternalInput").ap()
    # packed small weights: [p, t, 0:512]=Wh rows, [..,512:516]=hidden.T,
    # [..,516]=b, [..,517]=v   (host-packed so partition lines are contiguous)
    sm_d = nc.dram_tensor("sm", [P, N_HT * SM_C], F32, kind="ExternalInput").ap()
    out = nc.dram_tensor("out", [B_LOC, S], F32, kind="ExternalOutput").ap()

    with tile.TileContext(nc) as tc:
        with (
            tc.tile_pool(name="consts", bufs=1) as consts,
            tc.tile_pool(name="encTp", bufs=B_LOC) as encTp,
            tc.tile_pool(name="energyp", bufs=8) as energyp,
            tc.tile_pool(name="smp", bufs=2) as smp,
            tc.tile_pool(name="tpps", bufs=1, space="PSUM") as tpps,
            tc.tile_pool(name="outps", bufs=5, space="PSUM") as outps,
            tc.tile_pool(name="scps", bufs=2, space="PSUM") as scps,
        ):
            ident = consts.tile([P, P], F32)
            make_identity(nc, ident[:])

            # ---- DMAs: interleave We/enc0 per-e-slice so the first matmul
            # can start after ~2 slices instead of the whole 3MB stream.
            We_r = consts.tile([P, N_ET, H], BF16, name="We_r")
            encT = [
                encTp.tile([P, N_ET, S], BF16, tag="encT", name=f"encT{b}")
                for b in range(B_LOC)
            ]
            for j in range(N_ET):
                nc.sync.dma_start(We_r[:, j, :], We_d[:, j * H:(j + 1) * H])
                nc.sync.dma_start(encT[0][:, j, :], enc[0, :, j * S:(j + 1) * S])

            sm_sb = consts.tile([P, N_HT, SM_C], F32)
            nc.sync.dma_start(sm_sb[:], sm_d.rearrange("e (t c) -> e t c", t=N_HT))

            for b in range(1, B_LOC):
                nc.sync.dma_start(
                    encT[b][:], enc[b].rearrange("p (j s) -> p j s", j=N_ET)
                )

            Wh_sb = sm_sb[:, :, :H]
            hT_sb = sm_sb[:, :, H:H + B_LOC]
            b_sb = sm_sb[:, :, H + B_LOC]
            v_sb = sm_sb[:, :, H + B_LOC + 1]
            v_r = consts.tile([P, N_HT], F32R)
            nc.vector.tensor_copy(v_r[:], v_sb)
            hT_r = consts.tile([P, N_HT, B_LOC], F32R)
            nc.vector.tensor_copy(hT_r[:], hT_sb)
            Wh_r = consts.tile([P, N_HT, H], F32R)
            nc.vector.tensor_copy(Wh_r[:], Wh_sb)

            bias_sb = consts.tile([P, N_HT, B_LOC], F32)

            def emit_bias_setup():
                # hproj as [b, h] wide-N matmul, then PE-transpose to [h, b];
                # bias[h, b] = hproj[h, b] + b[h]
                ps_h = tpps.tile([B_LOC, H], F32, tag="tstage", name="ps_h")
                for j in range(N_HT):
                    nc.tensor.matmul(
                        ps_h[:],
                        hT_r[:, j, :],
                        Wh_r[:, j, :],
                        start=(j == 0),
                        stop=(j == N_HT - 1),
                    )
                hp_sb = consts.tile([B_LOC, H], F32, name="hp_sb")
                nc.vector.tensor_copy(hp_sb[:], ps_h[:])
                for i in range(N_HT):
                    tp_i = tpps.tile([P, B_LOC], F32, tag="tstage", name=f"tp_i{i}")
                    nc.tensor.transpose(
                        tp_i[:], hp_sb[:, i * P:(i + 1) * P], ident[:B_LOC, :B_LOC]
                    )
                    nc.vector.tensor_scalar_add(
                        bias_sb[:, i, :], tp_i[:], b_sb[:, i:i + 1]
                    )

            # ---- main loop: mains(c) emitted ahead, tail(c-1) after, so the
            # PE streams matmuls without waiting on the ScalarE tanh.
            chunks = [(bi, sc) for bi in range(B_LOC) for sc in range(N_SC)]
            probs_all = consts.tile([1, B_LOC * S], F32, name="probs_all")
            scores_sb = {}
            psums = {}
            score_ps = {}

            def emit_mains(ci):
                bi, sc = chunks[ci]
                s0 = sc * 512
                ps = [
                    outps.tile([P, 512], F32, tag="mmout", name=f"mmout{ci}_{i}")
                    for i in range(N_HT)
                ]
                psums[ci] = ps
                if ci == 0:
                    # j-outer: consume the per-j DMA slices as they arrive
                    for j in range(N_ET):
                        for i in range(N_HT):
                            nc.tensor.matmul(
                                ps[i][:],
                                We_r[:, j, i * P:(i + 1) * P],
                                encT[bi][:, j, s0:s0 + 512],
                                start=(j == 0),
                                stop=(j == N_ET - 1),
                            )
                else:
                    # i-outer: each PSUM group completes early so its tanh
                    # frees the bank well before it is reused
                    for i in range(N_HT):
                        for j in range(N_ET):
                            nc.tensor.matmul(
                                ps[i][:],
                                We_r[:, j, i * P:(i + 1) * P],
                                encT[bi][:, j, s0:s0 + 512],
                                start=(j == 0),
                                stop=(j == N_ET - 1),
                            )

            def emit_tail(ci):
                bi, sc = chunks[ci]
                s0 = sc * 512
                if sc == 0:
                    scores_sb[bi] = smp.tile(
                        [1, S], F32, tag="scores", name=f"scores{bi}"
                    )
                sc_ps = scps.tile([1, 512], F32, tag="scores_ps")
                for i in range(N_HT):
                    en = energyp.tile([P, 512], F32R, tag="energy", name=f"en{ci}_{i}")
                    nc.scalar.activation(
                        en[:],
                        psums[ci][i][:],
                        AFT.Tanh,
                        bias=bias_sb[:, i, bi:bi + 1],
                    )
                    nc.tensor.matmul(
                        sc_ps[:],
                        v_r[:, i:i + 1],
                        en[:],
                        start=(i == 0),
                        stop=(i == N_HT - 1),
                    )
                nc.vector.tensor_copy(scores_sb[bi][:, s0:s0 + 512], sc_ps[:])
                if sc == N_SC - 1:
                    # ---- softmax over s (partition 0) ----
                    negmax = smp.tile([1, 1], F32, tag="negmax")
                    nc.vector.reduce_max(
                        out=negmax[:], in_=scores_sb[bi][:], axis=AXX, negate=True
                    )
                    exp_sb = smp.tile([1, S], F32, tag="exp")
                    ssum = smp.tile([1, 1], F32, tag="ssum")
                    nc.scalar.activation(
                        exp_sb[:], scores_sb[bi][:], AFT.Exp, bias=negmax[:],
                        accum_out=ssum[:],
                    )
                    rec = smp.tile([1, 1], F32, tag="rec")
                    nc.vector.reciprocal(rec[:], ssum[:])
                    nc.vector.tensor_scalar_mul(
                        probs_all[:, bi * S:(bi + 1) * S], exp_sb[:], rec[:]
                    )

            for ci in range(len(chunks)):
                emit_mains(ci)
                if ci == 1:
                    # placed here so the slow sm-DMA -> cast chain it depends
                    # on never blocks the chunk-0/1 PE work
                    emit_bias_setup()
                if ci >= 1:
                    emit_tail(ci - 1)
            emit_tail(len(chunks) - 1)

            nc.sync.dma_start(
                out[:, :], probs_all[:].rearrange("p (b s) -> p b s", b=B_LOC)
            )

    nc.compile()
    return nc


_NC_CACHE = None


def _get_nc():
    global _NC_CACHE
    if _NC_CACHE is None:
        _NC_CACHE = build()
    return _NC_CACHE


def run(inputs, trace=False, trace_kwargs=None):
    hidden = np.ascontiguousarray(np.asarray(inputs["hidden"], dtype=np.float32))
    enc = np.asarray(inputs["encoder_outputs"], dtype=np.float32)
    W = np.ascontiguousarray(np.asarray(inputs["W"], dtype=np.float32))
    b = np.ascontiguousarray(np.asarray(inputs["b"], dtype=np.float32))
    v = np.ascontiguousarray(np.asarray(inputs["v"], dtype=np.float32))

    # enc: [B, S, E2] f32 -> bf16, transposed+packed to [B, 128, N_ET*S] with
    # enc_pk[b, p, j*S+s] = enc[b, s, j*128+p] (16KB-contiguous partition lines)
    enc_bf = enc.astype(ml_dtypes.bfloat16)
    enc_pk = np.ascontiguousarray(
        enc_bf.transpose(0, 2, 1)              # [B, E2, S]
        .reshape(B, N_ET, P, S)                # e -> (j, p)
        .transpose(0, 2, 1, 3)                 # [B, P, N_ET, S]
        .reshape(B, P, N_ET * S)
    )
    # We: [E2, H] -> bf16 packed [128, N_ET*H] with We_pk[p, j*H+h] = We[j*128+p, h]
    We = W[H:].astype(ml_dtypes.bfloat16)
    We_pk = np.ascontiguousarray(
        We.reshape(N_ET, P, H).transpose(1, 0, 2).reshape(P, N_ET * H)
    )

    nc = _get_nc()
    in_maps = []
    for c in range(N_CORES):
        lo, hi = c * B_LOC, (c + 1) * B_LOC
        sm = np.zeros((N_HT, P, SM_C), dtype=np.float32)
        sm[:, :, :H] = W[:H].reshape(N_HT, P, H)
        sm[:, :, H:H + B_LOC] = hidden[lo:hi].T.reshape(N_HT, P, B_LOC)
        sm[:, :, H + B_LOC] = b.reshape(N_HT, P)
        sm[:, :, H + B_LOC + 1] = v.reshape(N_HT, P)
        sm_pk = np.ascontiguousarray(
            sm.transpose(1, 0, 2).reshape(P, N_HT * SM_C)
        )
        in_maps.append(
            {
                "enc": enc_pk[lo:hi],
                "We": We_pk,
                "sm": sm_pk,
            }
        )
    res = bass_utils.run_bass_kernel_spmd(
        nc,
        in_maps,
        core_ids=list(range(N_CORES)),
        trace=trace,
        **(trace_kwargs or {}),
    )
    full = np.concatenate([res.results[c]["out"] for c in range(N_CORES)], axis=0)
    return full, res


def kernel(**inputs) -> np.ndarray:
    full, _ = run(inputs, trace=False)
    return full


# revision 4
# speedup vs baseline: 1.2530x; 1.2530x over previous
"""Bass/Tile TRN2 kernel for nn_Attention (Bahdanau-style attention scores).

Computation (per batch b):
    energy[s, h] = tanh( (enc[b] @ We)[s, h] + (hidden[b] @ Wh)[h] + bias[h] )
    scores[s]    = sum_h energy[s, h] * v[h]
    out[b]       = softmax(scores)

Sharding: data-parallel over batch B=32 across 8 cores (4 batches/core);
W, b, v replicated.

Per-core device program:
  - enc is transposed to [e, s] layout ON THE HOST and packed so every
    SBUF partition line is one contiguous DMA descriptor (the old
    device-side DMA-transpose was the bottleneck: 32768 x 256B descriptors
    kept the DMA engines 95% busy and paced the whole kernel).
  - main matmul We-tile @ encT in bf16 (216ns/512-col steady cadence),
    output layout [h, s] so the (h@Wh + b) bias is a per-partition scalar
    fused into the ScalarE tanh. PSUM groups are PAIRWISE interleaved:
    back-to-back accumulation into a single PSUM bank costs +46ns/matmul
    (measured), so groups alternate between two banks, and the first pair
    stops mid-chunk so its banks recycle without stalling the next chunk.
  - tanh+v-dot tails run one chunk behind the main matmuls, keeping the
    PE stream unbroken; v-dot is a k=h matmul with v as [128,1] stationary.
  - bias setup computes h_projT directly as [h,b]-output matmuls
    (stationary=Wh tile, moving=hidden^T, 4-col ap) - no PE transposes.
  - softmax over s on partition 0 (reduce_max -> exp with fused sum -> mul).
"""

import ml_dtypes
import numpy as np

import concourse.bass as bass
import concourse.tile as tile
from concourse import bacc, mybir
from concourse import bass_utils

F32 = mybir.dt.float32
F32R = mybir.dt.float32r
BF16 = mybir.dt.bfloat16
AFT = mybir.ActivationFunctionType
AXX = mybir.AxisListType.X

N_CORES = 8
B = 32
B_LOC = B // N_CORES  # 4
S = 1024
H = 512
E2 = 2 * H  # 1024
P = 128
N_HT = H // P   # 4 h-tiles
N_ET = E2 // P  # 8 e-tiles
N_SC = S // 512  # 2 s-chunks of 512
SM_C = H + B_LOC + 2


def build():
    nc = bacc.Bacc("TRN2", target_bir_lowering=False, debug=False)
    # host-pretransposed/packed: enc[b, p, j*S + s] = enc_orig[b, s, j*128+p]
    enc = nc.dram_tensor("enc", [B_LOC, P, N_ET * S], BF16, kind="ExternalInput").ap()
    # host-packed: We[p, j*H + h] = We_orig[j*128+p, h]
    We_d = nc.dram_tensor("We", [P, N_ET * H], BF16, kind="ExternalInput").ap()
    # packed small weights: [p, t, 0:512]=Wh rows, [..,512:516]=hidden.T,
    # [..,516]=b, [..,517]=v   (host-packed so partition lines are contiguous)
    sm_d = nc.dram_tensor("sm", [P, N_HT * SM_C], F32, kind="ExternalInput").ap()
    out = nc.dram_tensor("out", [B_LOC, S], F32, kind="ExternalOutput").ap()

    with tile.TileContext(nc) as tc:
        with (
            tc.tile_pool(name="consts", bufs=1) as consts,
            tc.tile_pool(name="encTp", bufs=B_LOC - 1) as encTp,
            tc.tile_pool(name="energyp", bufs=8) as energyp,
            tc.tile_pool(name="smp", bufs=2) as smp,
            tc.tile_pool(name="tpps", bufs=2, space="PSUM") as tpps,
            tc.tile_pool(name="outps", bufs=5, space="PSUM") as outps,
            tc.tile_pool(name="scps", bufs=1, space="PSUM") as scps,
        ):
            # ---- DMAs. sm first (tiny; its DVE-copy chain feeds the bias
            # setup ~8us in). Then We/enc0 interleaved per e-slice as
            # SEPARATE tiles so each chunk-0 matmul group waits only on its
            # own slice's semaphore, not the whole stream.
            sm_sb = consts.tile([P, N_HT, SM_C], F32)
            nc.sync.dma_start(sm_sb[:], sm_d.rearrange("e (t c) -> e t c", t=N_HT))

            We_j = []
            enc0 = {}  # (j, sc) -> [P, 512] tile
            for j in range(N_ET):
                w = consts.tile([P, H], BF16, name=f"We_j{j}")
                nc.sync.dma_start(w[:], We_d[:, j * H:(j + 1) * H])
                We_j.append(w)
                t = consts.tile([P, 512], BF16, name=f"enc0_{j}a")
                nc.sync.dma_start(t[:], enc[0, :, j * S:j * S + 512])
                enc0[(j, 0)] = t
            for j in range(N_ET):
                t = consts.tile([P, 512], BF16, name=f"enc0_{j}b")
                nc.sync.dma_start(t[:], enc[0, :, j * S + 512:(j + 1) * S])
                enc0[(j, 1)] = t

            encT = {0: None}
            for bi in range(1, B_LOC):
                t = encTp.tile([P, N_ET, S], BF16, tag="encT", name=f"encT{bi}")
                nc.sync.dma_start(t[:], enc[bi].rearrange("p (j s) -> p j s", j=N_ET))
                encT[bi] = t

            def moving(bi, j, sc):
                if bi == 0:
                    return enc0[(j, sc)][:]
                return encT[bi][:, j, sc * 512:sc * 512 + 512]

            Wh_sb = sm_sb[:, :, :H]
            hT_sb = sm_sb[:, :, H:H + B_LOC]
            b_sb = sm_sb[:, :, H + B_LOC]
            v_sb = sm_sb[:, :, H + B_LOC + 1]
            v_r = consts.tile([P, N_HT], F32R)
            nc.vector.tensor_copy(v_r[:], v_sb)
            hT_r = consts.tile([P, N_HT, B_LOC], F32R)
            nc.vector.tensor_copy(hT_r[:], hT_sb)
            Wh_r = consts.tile([P, N_HT, H], F32R)
            nc.vector.tensor_copy(Wh_r[:], Wh_sb)

            bias_sb = consts.tile([P, N_HT, B_LOC], F32)

            def emit_bias_setup():
                # h_projT[h_out, b] tile i: accumulate over h_in tiles j with
                # stationary Wh[h_in, h_out-slice] and moving hidden^T[h_in, b].
                # 4-wide moving -> ~16 tiny matmuls, no transposes, and the
                # DVE bias-add reads PSUM directly without blocking the PE.
                for i in range(N_HT):
                    tp_i = tpps.tile([P, B_LOC], F32, tag="tstage", name=f"tp_i{i}")
                    for j in range(N_HT):
                        nc.tensor.matmul(
                            tp_i[:],
                            Wh_r[:, j, i * P:(i + 1) * P],
                            hT_r[:, j, :],
                            start=(j == 0),
                            stop=(j == N_HT - 1),
                        )
                    nc.vector.tensor_scalar_add(
                        bias_sb[:, i, :], tp_i[:], b_sb[:, i:i + 1]
                    )

            # ---- main loop: mains(c) emitted ahead, tail(c-1) after, so the
            # PE streams matmuls without waiting on the ScalarE tanh.
            chunks = [(bi, sc) for bi in range(B_LOC) for sc in range(N_SC)]
            probs_all = consts.tile([1, B_LOC * S], F32, name="probs_all")
            scores_sb = {}
            psums = {}

            def emit_mains(ci):
                bi, sc = chunks[ci]
                ps = [
                    outps.tile([P, 512], F32, tag="mmout", name=f"mmout{ci}_{i}")
                    for i in range(N_HT)
                ]
                psums[ci] = ps
                for pair in range(2):
                    i0, i1 = 2 * pair, 2 * pair + 1
                    for j in range(N_ET):
                        nc.tensor.matmul(
                            ps[i0][:],
                            We_j[j][:, i0 * P:(i0 + 1) * P],
                            moving(bi, j, sc),
                            start=(j == 0),
                            stop=(j == N_ET - 1),
                        )
                        nc.tensor.matmul(
                            ps[i1][:],
                            We_j[j][:, i1 * P:(i1 + 1) * P],
                            moving(bi, j, sc),
                            start=(j == 0),
                            stop=(j == N_ET - 1),
                        )

            def emit_tail(ci):
                bi, sc = chunks[ci]
                s0 = sc * 512
                if sc == 0:
                    scores_sb[bi] = smp.tile(
                        [1, S], F32, tag="scores", name=f"scores{bi}"
                    )
                sc_ps = scps.tile([1, 512], F32, tag="scores_ps")
                for i in range(N_HT):
                    en = energyp.tile([P, 512], F32R, tag="energy", name=f"en{ci}_{i}")
                    nc.scalar.activation(
                        en[:],
                        psums[ci][i][:],
                        AFT.Tanh,
                        bias=bias_sb[:, i, bi:bi + 1],
                    )
                    nc.tensor.matmul(
                        sc_ps[:],
                        v_r[:, i:i + 1],
                        en[:],
                        start=(i == 0),
                        stop=(i == N_HT - 1),
                    )
                nc.vector.tensor_copy(scores_sb[bi][:, s0:s0 + 512], sc_ps[:])
                if sc == N_SC - 1:
                    # ---- softmax over s (partition 0), then stream this
                    # batch's 4KB of output immediately ----
                    negmax = smp.tile([1, 1], F32, tag="negmax")
                    nc.vector.reduce_max(
                        out=negmax[:], in_=scores_sb[bi][:], axis=AXX, negate=True
                    )
                    exp_sb = smp.tile([1, S], F32, tag="exp")
                    ssum = smp.tile([1, 1], F32, tag="ssum")
                    nc.scalar.activation(
                        exp_sb[:], scores_sb[bi][:], AFT.Exp, bias=negmax[:],
                        accum_out=ssum[:],
                    )
                    rec = smp.tile([1, 1], F32, tag="rec")
                    nc.vector.reciprocal(rec[:], ssum[:])
                    nc.vector.tensor_scalar_mul(
                        probs_all[:, bi * S:(bi + 1) * S], exp_sb[:], rec[:]
                    )
                    nc.sync.dma_start(
                        out[bi:bi + 1, :].rearrange("b s -> () (b s)"),
                        probs_all[:, bi * S:(bi + 1) * S],
                    )

            for ci in range(len(chunks)):
                emit_mains(ci)
                if ci == 1:
                    emit_bias_setup()
                if ci >= 1:
                    emit_tail(ci - 1)
            emit_tail(len(chunks) - 1)

    nc.compile()
    return nc


_NC_CACHE = None


def _get_nc():
    global _NC_CACHE
    if _NC_CACHE is None:
        _NC_CACHE = build()
    return _NC_CACHE


def run(inputs, trace=False, trace_kwargs=None):
    hidden = np.ascontiguousarray(np.asarray(inputs["hidden"], dtype=np.float32))
    enc = np.asarray(inputs["encoder_outputs"], dtype=np.float32)
    W = np.ascontiguousarray(np.asarray(inputs["W"], dtype=np.float32))
    b = np.ascontiguousarray(np.asarray(inputs["b"], dtype=np.float32))
    v = np.ascontiguousarray(np.asarray(inputs["v"], dtype=np.float32))

    # enc: [B, S, E2] f32 -> bf16, transposed+packed to [B, 128, N_ET*S] with
    # enc_pk[b, p, j*S+s] = enc[b, s, j*128+p] (16KB-contiguous partition lines)
    enc_bf = enc.astype(ml_dtypes.bfloat16)
    enc_pk = np.ascontiguousarray(
        enc_bf.transpose(0, 2, 1)              # [B, E2, S]
        .reshape(B, N_ET, P, S)                # e -> (j, p)
        .transpose(0, 2, 1, 3)                 # [B, P, N_ET, S]
        .reshape(B, P, N_ET * S)
    )
    # We: [E2, H] -> bf16 packed [128, N_ET*H] with We_pk[p, j*H+h] = We[j*128+p, h]
    We = W[H:].astype(ml_dtypes.bfloat16)
    We_pk = np.ascontiguousarray(
        We.reshape(N_ET, P, H).transpose(1, 0, 2).reshape(P, N_ET * H)
    )

    nc = _get_nc()
    in_maps = []
    for c in range(N_CORES):
        lo, hi = c * B_LOC, (c + 1) * B_LOC
        sm = np.zeros((N_HT, P, SM_C), dtype=np.float32)
        sm[:, :, :H] = W[:H].reshape(N_HT, P, H)
        sm[:, :, H:H + B_LOC] = hidden[lo:hi].T.reshape(N_HT, P, B_LOC)
        sm[:, :, H + B_LOC] = b.reshape(N_HT, P)
        sm[:, :, H + B_LOC + 1] = v.reshape(N_HT, P)
        sm_pk = np.ascontiguousarray(
            sm.transpose(1, 0, 2).reshape(P, N_HT * SM_C)
        )
        in_maps.append(
            {
                "enc": enc_pk[lo:hi],
                "We": We_pk,
                "sm": sm_pk,
            }
        )
    res = bass_utils.run_bass_kernel_spmd(
        nc,
        in_maps,
        core_ids=list(range(N_CORES)),
        trace=trace,
        **(trace_kwargs or {}),
    )
    full = np.concatenate([res.results[c]["out"] for c in range(N_CORES)], axis=0)
    return full, res


def kernel(**inputs) -> np.ndarray:
    full, _ = run(inputs, trace=False)
    return full
